# revision 6
# baseline (speedup 1.0000x reference)
"""Trainium2 Bass kernel for the Poisson PINN problem.

Computes (d2u/dx2 at equation points, u at boundary points) for a
[1,256,256,256,256,1] tanh MLP, via exact forward Taylor-mode propagation
of (value, first, second derivative) streams — no autodiff graphs.

Data-parallel over 8 NeuronCores: x_equation/x_boundary sharded along the
batch axis, MLP weights replicated. No collectives.

Per-core program (feature-major layout, fp16 streams, fp32 accumulation):
  - activations [128 partitions = feature-half, free = 2 tiles x 2 halves
    x 512 batch]; batch tiles of 512 processed in pairs so SBUF-only
    elementwise ops run at free-dim 2048.
  - hidden matmuls on TensorE (fp16, K/M=128 blocks, N=512, PSUM fp32).
  - tanh/Square/PSUM-casts on ScalarE; derivative-chain products on
    VectorE using only tensor_tensor/tensor_scalar class ops (2x/4x modes).
  - derivative streams carry folded constants (D2 = -0.5*h'') so the
    chain needs no extra scaling ops; host passes W4n = -2*W4 to undo.
"""

import sys

import numpy as np

for _p in ("/opt/trn_rl_repo", "/root/.axon_site/_ro/trn_rl_repo"):
    if _p not in sys.path:
        sys.path.append(_p)

import concourse.bass as bass  # noqa: E402
import concourse.mybir as mybir  # noqa: E402
import concourse.tile as tile  # noqa: E402
from concourse import bacc  # noqa: E402
from concourse.bass_utils import run_bass_kernel_spmd  # noqa: E402

f32 = mybir.dt.float32
f16 = mybir.dt.float16
AF = mybir.ActivationFunctionType
OP = mybir.AluOpType

N_EQ = 262144
N_B = 8192
N_CORES = 8
F = 256          # hidden width
H = 128          # feature half (partition dim)
BT = 512         # batch tile (one fp32 PSUM bank)

_program_cache = {}


def _build_program(neq_core, nb_core, zero_bias):
    assert neq_core % (2 * BT) == 0 and nb_core % (2 * BT) == 0
    n_eq_pairs = neq_core // (2 * BT)
    n_b_pairs = nb_core // (2 * BT)

    nc = bacc.Bacc("TRN2", target_bir_lowering=False, debug=False,
                   num_devices=N_CORES)

    x_eq = nc.dram_tensor("x_eq", [neq_core], f32, kind="ExternalInput")
    x_b = nc.dram_tensor("x_b", [nb_core], f32, kind="ExternalInput")
    wd = {l: nc.dram_tensor(f"w{l}", [F, F], f16, kind="ExternalInput")
          for l in (1, 2, 3)}
    wdf = {l: nc.dram_tensor(f"w{l}f", [F, F], f32, kind="ExternalInput")
          for l in (1, 2, 3)}
    w4p = nc.dram_tensor("w4p", [H, 2], f32, kind="ExternalInput")
    w4n = nc.dram_tensor("w4n", [H, 2], f16, kind="ExternalInput")
    scal_names = ["w0", "negw0", "m2w0", "b0", "b1", "b2", "b3"]
    scald = {s: nc.dram_tensor(s, [H, 2], f32, kind="ExternalInput")
             for s in scal_names}
    b4 = nc.dram_tensor("b4", [1, 1], f32, kind="ExternalInput")

    out_eq = nc.dram_tensor("out_eq", [neq_core], f32, kind="ExternalOutput")
    out_b = nc.dram_tensor("out_b", [nb_core], f32, kind="ExternalOutput")

    with tile.TileContext(nc) as tc:
        with (
            tc.tile_pool(name="wpool", bufs=1) as wpool,
            tc.tile_pool(name="xpool", bufs=3) as xpool,
            tc.tile_pool(name="hpool", bufs=2) as hpool,
            tc.tile_pool(name="dpool", bufs=2) as dpool,
            tc.tile_pool(name="tpool", bufs=2) as tpool,
            tc.tile_pool(name="opool", bufs=4) as opool,
            tc.tile_pool(name="zpsum", bufs=2, space="PSUM") as zpsum,
            tc.tile_pool(name="zppsum", bufs=1, space="PSUM") as zppsum,
            tc.tile_pool(name="zpppsum", bufs=1, space="PSUM") as zpppsum,
        ):
            W = {}
            for l in (1, 2, 3):
                for k in (0, 1):
                    t = wpool.tile([H, F], f16, tag=f"w{l}_{k}", name=f"w{l}{k}")
                    nc.sync.dma_start(out=t[:], in_=wd[l][k * H:(k + 1) * H, :])
                    W[l, k] = t
            WF = {}
            for l in (1, 2, 3):
                for k2 in (0, 1):
                    t = wpool.tile([H, F], f32, tag=f"wf{l}_{k2}",
                                   name=f"wf{l}{k2}")
                    nc.sync.dma_start(out=t[:],
                                      in_=wdf[l][k2 * H:(k2 + 1) * H, :])
                    WF[l, k2] = t
            w4p_t = wpool.tile([H, 2], f32, tag="w4p")
            nc.sync.dma_start(out=w4p_t[:], in_=w4p[:])
            w4n_t = wpool.tile([H, 2], f16, tag="w4n")
            nc.sync.dma_start(out=w4n_t[:], in_=w4n[:])
            S = {}
            for s in scal_names:
                t = wpool.tile([H, 2], f32, tag=s, name=f"sc_{s}")
                nc.sync.dma_start(out=t[:], in_=scald[s][:])
                S[s] = t
            b4_t = wpool.tile([1, 1], f32, tag="b4")
            nc.sync.dma_start(out=b4_t[:], in_=b4[:])

            def scal(name, h):
                return S[name][:, h:h + 1]

            def jcols(j, h):
                return slice((2 * j + h) * BT, (2 * j + h + 1) * BT)

            def jspan(j):
                return slice(2 * j * BT, (2 * j + 2) * BT)

            def pcols(h):
                return slice(h * BT, (h + 1) * BT)

            def mlp_pair(x_dram, i0, deriv, o_dram, sq_on_act):
                xb = [xpool.tile([H, BT], f32, tag="xb", name=f"xb{j}")
                      for j in (0, 1)]
                for j in (0, 1):
                    src = x_dram[i0 + j * BT:i0 + (j + 1) * BT]
                    nc.sync.dma_start(
                        out=xb[j][:],
                        in_=src.unsqueeze(0).partition_broadcast(H))

                # layer 0: z0 = w0*x + b0 fused into the ACT scale/bias
                t = hpool.tile([H, 4 * BT], f16, tag="h")
                for j in (0, 1):
                    for h in (0, 1):
                        nc.scalar.activation(t[:, jcols(j, h)], xb[j][:],
                                             AF.Tanh, bias=scal("b0", h),
                                             scale=scal("w0", h))
                if deriv:
                    sq = tpool.tile([H, 4 * BT], f16, tag="sq")
                    nc.scalar.activation(sq[:], t[:], AF.Square)
                    d1 = dpool.tile([H, 4 * BT], f16, tag="d1")
                    for j in (0, 1):
                        for h in (0, 1):
                            # (sq * -w0) + w0 = w0*(1-t^2) = h0'
                            nc.vector.tensor_scalar(
                                d1[:, jcols(j, h)], sq[:, jcols(j, h)],
                                scal("negw0", h), scal("w0", h),
                                OP.mult, OP.add)
                    q = tpool.tile([H, 4 * BT], f16, tag="q")
                    nc.vector.tensor_tensor(q[:], t[:], d1[:], OP.mult)
                    d2 = dpool.tile([H, 4 * BT], f16, tag="d2")
                    for j in (0, 1):
                        for h in (0, 1):
                            # q * (-2 w0) = h0''
                            nc.vector.tensor_scalar(
                                d2[:, jcols(j, h)], q[:, jcols(j, h)],
                                scal("m2w0", h), None, OP.mult)

                for l in (1, 2, 3):
                    z = [zpsum.tile([H, 2 * BT], f32, tag="z", name=f"z{j}")
                         for j in (0, 1)]
                    zp = zpp = None
                    if deriv:
                        zp = [zppsum.tile([H, 2 * BT], f32, tag="zp",
                                          name=f"zp{j}") for j in (0, 1)]
                        zpp = [zpppsum.tile([H, 2 * BT], f32, tag="zpp",
                                            name=f"zpp{j}") for j in (0, 1)]
                    for j in (0, 1):
                        for h in (0, 1):
                            for k in (0, 1):
                                nc.tensor.matmul(
                                    z[j][:, pcols(h)],
                                    lhsT=W[l, k][:, h * H:(h + 1) * H],
                                    rhs=t[:, jcols(j, k)],
                                    start=(k == 0), stop=(k == 1))
                        if deriv:
                            for h in (0, 1):
                                for k in (0, 1):
                                    nc.tensor.matmul(
                                        zp[j][:, pcols(h)],
                                        lhsT=W[l, k][:, h * H:(h + 1) * H],
                                        rhs=d1[:, jcols(j, k)],
                                        start=(k == 0), stop=(k == 1))
                            for h in (0, 1):
                                for k in (0, 1):
                                    nc.tensor.matmul(
                                        zpp[j][:, pcols(h)],
                                        lhsT=W[l, k][:, h * H:(h + 1) * H],
                                        rhs=d2[:, jcols(j, k)],
                                        start=(k == 0), stop=(k == 1))
                    tn = hpool.tile([H, 4 * BT], f16, tag="h")
                    for j in (0, 1):
                        if zero_bias:
                            nc.scalar.activation(tn[:, jspan(j)], z[j][:],
                                                 AF.Tanh)
                        else:
                            for h in (0, 1):
                                nc.scalar.activation(
                                    tn[:, jcols(j, h)], z[j][:, pcols(h)],
                                    AF.Tanh, bias=scal(f"b{l}", h))
                    if deriv:
                        czp = tpool.tile([H, 4 * BT], f16, tag="czp")
                        czpp = tpool.tile([H, 4 * BT], f16, tag="czpp")
                        for j in (0, 1):
                            nc.scalar.activation(czp[:, jspan(j)], zp[j][:],
                                                 AF.Copy)
                            nc.scalar.activation(czpp[:, jspan(j)], zpp[j][:],
                                                 AF.Copy)
                        sq = tpool.tile([H, 4 * BT], f16, tag="sq")
                        if sq_on_act:
                            nc.scalar.activation(sq[:], tn[:], AF.Square)
                        else:
                            nc.vector.tensor_tensor(sq[:], tn[:], tn[:],
                                                    OP.mult)
                        s = tpool.tile([H, 4 * BT], f16, tag="s")
                        nc.vector.tensor_scalar(s[:], sq[:], -1.0, 1.0,
                                                OP.mult, OP.add)
                        d1n = dpool.tile([H, 4 * BT], f16, tag="d1")
                        nc.vector.tensor_tensor(d1n[:], s[:], czp[:], OP.mult)
                        cl = -0.5 if l == 1 else 1.0
                        a = tpool.tile([H, 4 * BT], f16, tag="a")
                        if cl == 1.0:
                            nc.vector.tensor_tensor(a[:], s[:], czpp[:],
                                                    OP.mult)
                        else:
                            nc.vector.scalar_tensor_tensor(
                                a[:], czpp[:], cl, s[:], OP.mult, OP.mult)
                        m = tpool.tile([H, 4 * BT], f16, tag="m")
                        nc.vector.tensor_tensor(m[:], tn[:], czp[:], OP.mult)
                        u = tpool.tile([H, 4 * BT], f16, tag="u")
                        nc.vector.tensor_tensor(u[:], m[:], d1n[:], OP.mult)
                        d2n = dpool.tile([H, 4 * BT], f16, tag="d2")
                        nc.vector.tensor_tensor(d2n[:], a[:], u[:], OP.add)
                        d1, d2 = d1n, d2n
                    t = tn

                for j in (0, 1):
                    up = zpppsum.tile([1, BT], f32, tag="zpp", name=f"up{j}")
                    src = d2 if deriv else t
                    wt = w4n_t if deriv else w4p_t
                    for k in (0, 1):
                        nc.tensor.matmul(up[:], lhsT=wt[:, k:k + 1],
                                         rhs=src[:, jcols(j, k)],
                                         start=(k == 0), stop=(k == 1))
                    o = opool.tile([1, BT], f32, tag="o", name=f"o{j}")
                    if deriv:
                        nc.vector.tensor_scalar(o[:], up[:], 0.0, None, OP.add)
                    else:
                        nc.vector.tensor_scalar(o[:], up[:], b4_t[0:1, 0:1],
                                                None, OP.add)
                    nc.sync.dma_start(
                        out=o_dram[i0 + j * BT:i0 + (j + 1) * BT].unsqueeze(0),
                        in_=o[:])

            for i in range(n_eq_pairs):
                mlp_pair(x_eq, i * 2 * BT, True, out_eq,
                         sq_on_act=(i % 3 == 0))
            for i in range(n_b_pairs):
                mlp_pair(x_b, i * 2 * BT, False, out_b, sq_on_act=False)

    nc.finalize()
    return nc


def _get_program(neq_core, nb_core, zero_bias):
    key = (neq_core, nb_core, zero_bias)
    if key not in _program_cache:
        _program_cache[key] = _build_program(neq_core, nb_core, zero_bias)
    return _program_cache[key]


def _host_inputs(x_eq_shard, x_b_shard, Ws, bs):
    W0 = Ws[0][0].astype(np.float32)
    pack = lambda v: np.stack([v[:H], v[H:]], axis=1).astype(np.float32)
    m = {
        "x_eq": np.ascontiguousarray(x_eq_shard.reshape(-1).astype(np.float32)),
        "x_b": np.ascontiguousarray(x_b_shard.reshape(-1).astype(np.float32)),
        "w0": pack(W0),
        "negw0": pack(-W0),
        "m2w0": pack(-2.0 * W0),
        "b0": pack(bs[0].astype(np.float32)),
        "b1": pack(bs[1].astype(np.float32)),
        "b2": pack(bs[2].astype(np.float32)),
        "b3": pack(bs[3].astype(np.float32)),
        "b4": bs[4].astype(np.float32).reshape(1, 1),
        "w4p": np.stack([Ws[4][:H, 0], Ws[4][H:, 0]], axis=1).astype(np.float32),
        # -2*W4 undoes the folded -0.5 in the D2 stream
        "w4n": np.stack([-2.0 * Ws[4][:H, 0], -2.0 * Ws[4][H:, 0]],
                        axis=1).astype(np.float16),
    }
    for l in (1, 2, 3):
        m[f"w{l}"] = Ws[l].astype(np.float16)
        m[f"w{l}f"] = Ws[l].astype(np.float32)
    return m



def _install_profile_shim():
    """antenv.axon_hooks is missing from this image; shim it so
    run_bass_kernel_spmd(trace=True) can capture NTFF timing."""
    import types
    try:
        import antenv.axon_hooks  # noqa: F401
        return
    except ImportError:
        pass
    try:
        import antenv
        from trn_agent_boot.trn_boot import _ntff_profile_via_ctypes
        hook = _ntff_profile_via_ctypes('/opt/axon/libaxon_pjrt.so')
        mod = types.ModuleType("antenv.axon_hooks")
        mod.get_axon_ntff_profile_hook = lambda: hook
        mod.set_axon_ntff_profile_hook = lambda h: None
        sys.modules["antenv.axon_hooks"] = mod
        antenv.axon_hooks = mod
    except Exception:
        pass


def run(trace=False, tmpdir=None, **inputs):
    """Run on 8 NeuronCores; returns (d2u_dx2 [N_EQ,1], u_b [N_B,1], results)."""
    x_eq = np.asarray(inputs["x_equation"], dtype=np.float32).reshape(-1)
    x_b = np.asarray(inputs["x_boundary"], dtype=np.float32).reshape(-1)
    Ws = [np.asarray(inputs[f"W{i}"], dtype=np.float32) for i in range(5)]
    bs = [np.asarray(inputs[f"b{i}"], dtype=np.float32) for i in range(5)]
    assert x_eq.shape[0] == N_EQ and x_b.shape[0] == N_B

    neq_core = N_EQ // N_CORES
    nb_core = N_B // N_CORES
    zero_bias = all(not np.any(b) for b in bs[:4])
    nc = _get_program(neq_core, nb_core, zero_bias)

    in_maps = [
        _host_inputs(x_eq[c * neq_core:(c + 1) * neq_core],
                     x_b[c * nb_core:(c + 1) * nb_core], Ws, bs)
        for c in range(N_CORES)
    ]
    if trace:
        _install_profile_shim()
    res = run_bass_kernel_spmd(nc, in_maps, list(range(N_CORES)),
                               trace=trace, tmpdir=tmpdir)
    d2u = np.concatenate([res.results[c]["out_eq"] for c in range(N_CORES)])
    ub = np.concatenate([res.results[c]["out_b"] for c in range(N_CORES)])
    return d2u.reshape(N_EQ, 1).astype(np.float32), \
        ub.reshape(N_B, 1).astype(np.float32), res


def kernel(**inputs):
    d2u, ub, _ = run(trace=False, **inputs)
    return d2u, ub
